# revision 1
# baseline (speedup 1.0000x reference)
"""Two-layer GAT on 8 Trainium2 NeuronCores (Bass/Tile SPMD kernel).

Sharding: nodes are range-partitioned across the 8 cores; each core owns the
edges whose *destination* falls in its partition (segment softmax is per-dst,
so the softmax/aggregation is fully core-local). Per-layer node feature
tables are replicated: layer 1's table (h1 = x@W1 plus attention logits) is
computed redundantly on every core; layer 2's table is AllGathered after the
layer-1 aggregation.

Per-edge math uses the shift-invariance of softmax:
    out_d = sum_e exp(alpha_e) * h[src_e] / sum_e exp(alpha_e)
so a single pass over the edges suffices (no segment max needed; alpha is
O(1) here so exp cannot overflow).
"""

import sys

sys.path.insert(0, "/opt/trn_rl_repo")

import numpy as np

# ---------------------------------------------------------------------------
# configuration
# ---------------------------------------------------------------------------

FULL_CFG = dict(
    N=100000,      # real nodes
    IN_CH=512,     # input features
    HEADS=8,
    C=16,          # out channels per head
    NC=8,          # cores
)

NEG_SLOPE = 0.2
EPS = 1e-16


def _derive(cfg):
    d = dict(cfg)
    d["HC"] = d["HEADS"] * d["C"]                 # 128
    assert d["HC"] == 128
    assert d["IN_CH"] % 128 == 0
    d["KC"] = d["IN_CH"] // 128                   # k-chunks for x@W1
    assert d["N"] % d["NC"] == 0
    d["OWN"] = d["N"] // d["NC"]                  # real nodes per core
    d["BLK"] = (d["OWN"] + 127) // 128            # dst blocks per core
    d["OWN_PAD"] = d["BLK"] * 128
    d["NP"] = d["NC"] * d["OWN_PAD"]              # padded global nodes
    return d


# ---------------------------------------------------------------------------
# host-side prep: edge partitioning / padding / layouts
# ---------------------------------------------------------------------------

def _host_prep(cfg, x, edge_index, W1, att_src1, att_dst1, bias1, W2,
               att_src2, att_dst2, bias2):
    N, NC, OWN, BLK, OWN_PAD, NP, KC, H, C = (
        cfg["N"], cfg["NC"], cfg["OWN"], cfg["BLK"], cfg["OWN_PAD"],
        cfg["NP"], cfg["KC"], cfg["HEADS"], cfg["C"])
    HC = H * C

    src = np.asarray(edge_index[0], dtype=np.int64)
    dst = np.asarray(edge_index[1], dtype=np.int64)

    core = dst // OWN                       # owning core of each edge
    ldst = (dst - core * OWN).astype(np.int64)    # local dst id [0, OWN)
    srcp = ((src // OWN) * OWN_PAD + (src % OWN)).astype(np.int32)  # padded gid
    dstp = (core * OWN_PAD + ldst).astype(np.int32)
    blk = ldst // 128
    slot = ldst % 128

    # per-(core, block) edge counts -> shared tile counts T_b (SPMD uniform)
    counts = np.zeros((NC, BLK), dtype=np.int64)
    np.add.at(counts, (core, blk), 1)
    Tb = np.maximum(1, (counts.max(axis=0) + 127) // 128).astype(np.int64)
    off = np.zeros(BLK, dtype=np.int64)
    off[1:] = np.cumsum(Tb)[:-1]
    totT = int(Tb.sum())

    # per-core edge arrays, laid out [128, totT]:
    #   edge e of block b  ->  (partition p = e%128, column off[b] + e//128)
    srcp_a = np.zeros((NC, 128, totT), dtype=np.int32)
    dstp_a = np.zeros((NC, 128, totT), dtype=np.int32)
    dstl_a = np.zeros((NC, 128, totT), dtype=np.int32)
    slot_a = np.full((NC, 128, totT), 999.0, dtype=np.float32)

    order = np.lexsort((blk, core))
    s_core, s_blk = core[order], blk[order]
    s_srcp, s_dstp, s_ldst, s_slot = (
        srcp[order], dstp[order], ldst[order], slot[order])
    # position of each edge within its (core, block) group
    grp = s_core * BLK + s_blk
    first = np.ones(len(grp), dtype=bool)
    first[1:] = grp[1:] != grp[:-1]
    starts = np.flatnonzero(first)
    group_start = np.repeat(starts, np.diff(np.append(starts, len(grp))))
    pos = np.arange(len(grp)) - group_start
    pp = pos % 128
    col = off[s_blk] + pos // 128
    srcp_a[s_core, pp, col] = s_srcp
    dstp_a[s_core, pp, col] = s_dstp
    dstl_a[s_core, pp, col] = s_ldst
    slot_a[s_core, pp, col] = s_slot.astype(np.float32)

    # x in padded layout, pre-transposed for matmul lhsT:
    # xTb[p, kc, g] = x[orig(g), kc*128 + p]
    xp = np.zeros((NP, cfg["IN_CH"]), dtype=np.float32)
    for m in range(NC):
        xp[m * OWN_PAD:m * OWN_PAD + OWN] = x[m * OWN:(m + 1) * OWN]
    xTb = np.ascontiguousarray(
        xp.reshape(NP, KC, 128).transpose(2, 1, 0))  # [128, KC, NP]

    W1b = np.ascontiguousarray(
        np.asarray(W1, np.float32).reshape(KC, 128, HC).transpose(1, 0, 2))

    iota = np.broadcast_to(
        np.arange(128, dtype=np.float32)[None, :], (128, 128)).copy()
    attS1 = np.broadcast_to(
        np.asarray(att_src1, np.float32).reshape(1, HC), (128, HC)).copy()
    attD1 = np.broadcast_to(
        np.asarray(att_dst1, np.float32).reshape(1, HC), (128, HC)).copy()
    b1b = np.broadcast_to(
        np.asarray(bias1, np.float32).reshape(1, HC), (128, HC)).copy()
    W2f = np.asarray(W2, np.float32)
    va = np.einsum("khc,hc->kh", W2f.reshape(HC, H, C),
                   np.asarray(att_src2, np.float32))
    vd = np.einsum("khc,hc->kh", W2f.reshape(HC, H, C),
                   np.asarray(att_dst2, np.float32))
    vavd = np.ascontiguousarray(
        np.concatenate([va, vd], axis=1).astype(np.float32))  # [128, 2H]
    b2b = np.broadcast_to(
        np.asarray(bias2, np.float32).reshape(1, C), (128, C)).copy()

    shared = dict(xTb=xTb, W1b=W1b, iota=iota, attS1=attS1, attD1=attD1,
                  b1b=b1b, W2=W2f, vavd=vavd, b2b=b2b)
    in_maps = []
    for m in range(NC):
        im = dict(shared)
        im["srcp_a"] = np.ascontiguousarray(srcp_a[m])
        im["dstp_a"] = np.ascontiguousarray(dstp_a[m])
        im["dstl_a"] = np.ascontiguousarray(dstl_a[m])
        im["slot_a"] = np.ascontiguousarray(slot_a[m])
        in_maps.append(im)

    return in_maps, Tb.tolist(), off.tolist(), totT


# ---------------------------------------------------------------------------
# device program
# ---------------------------------------------------------------------------

def build_program(cfg, Tb, off, totT):
    from concourse import bacc, bass, mybir, tile
    from concourse.masks import make_identity

    f32 = mybir.dt.float32
    i32 = mybir.dt.int32
    X = mybir.AxisListType.X
    AF = mybir.ActivationFunctionType
    NC, NP, OWN_PAD, BLK, KC, H, C = (
        cfg["NC"], cfg["NP"], cfg["OWN_PAD"], cfg["BLK"], cfg["KC"],
        cfg["HEADS"], cfg["C"])
    HC = H * C
    NB_ALL = NP // 128       # node blocks in the full padded table
    W = 8 + HC               # table row width: [h (128) | a_src (8)]

    nc = bacc.Bacc("TRN2", target_bir_lowering=False, debug=False,
                   num_devices=NC)

    # inputs
    t_xTb = nc.dram_tensor("xTb", [128, KC, NP], f32, kind="ExternalInput")
    t_W1b = nc.dram_tensor("W1b", [128, KC, HC], f32, kind="ExternalInput")
    t_iota = nc.dram_tensor("iota", [128, 128], f32, kind="ExternalInput")
    t_attS1 = nc.dram_tensor("attS1", [128, HC], f32, kind="ExternalInput")
    t_attD1 = nc.dram_tensor("attD1", [128, HC], f32, kind="ExternalInput")
    t_b1b = nc.dram_tensor("b1b", [128, HC], f32, kind="ExternalInput")
    t_W2 = nc.dram_tensor("W2", [HC, HC], f32, kind="ExternalInput")
    t_vavd = nc.dram_tensor("vavd", [HC, 2 * H], f32, kind="ExternalInput")
    t_b2b = nc.dram_tensor("b2b", [128, C], f32, kind="ExternalInput")
    t_srcp = nc.dram_tensor("srcp_a", [128, totT], i32, kind="ExternalInput")
    t_dstp = nc.dram_tensor("dstp_a", [128, totT], i32, kind="ExternalInput")
    t_dstl = nc.dram_tensor("dstl_a", [128, totT], i32, kind="ExternalInput")
    t_slot = nc.dram_tensor("slot_a", [128, totT], f32, kind="ExternalInput")
    t_out = nc.dram_tensor("out", [OWN_PAD, C], f32, kind="ExternalOutput")

    def fv(ap, dims, extra_offset=0):
        """View `ap` with custom free-dim [step, count] pairs."""
        return bass.AP(ap.tensor, ap.offset + extra_offset, [ap.ap[0]] + dims)

    with tile.TileContext(nc) as tc:
        with (
            tc.tile_pool(name="const", bufs=1) as cpool,
            tc.tile_pool(name="dram", bufs=1, space="DRAM") as dram,
        ):
            # ---------------- constants ----------------
            W1_sb = cpool.tile([128, KC * HC], f32, tag="w1")
            nc.sync.dma_start(
                out=fv(W1_sb[:], [[HC, KC], [1, HC]]), in_=t_W1b[:, :, :])
            iota_sb = cpool.tile([128, 128], f32, tag="iota")
            nc.sync.dma_start(out=iota_sb[:], in_=t_iota[:, :])
            attS1_sb = cpool.tile([128, HC], f32, tag="attS")
            nc.sync.dma_start(out=attS1_sb[:], in_=t_attS1[:, :])
            attD1_sb = cpool.tile([128, HC], f32, tag="attD")
            nc.sync.dma_start(out=attD1_sb[:], in_=t_attD1[:, :])
            b1_sb = cpool.tile([128, HC], f32, tag="b1")
            nc.sync.dma_start(out=b1_sb[:], in_=t_b1b[:, :])
            W2_sb = cpool.tile([HC, HC], f32, tag="w2")
            nc.sync.dma_start(out=W2_sb[:], in_=t_W2[:, :])
            vavd_sb = cpool.tile([HC, 2 * H], f32, tag="vavd")
            nc.sync.dma_start(out=vavd_sb[:], in_=t_vavd[:, :])
            b2_sb = cpool.tile([128, C], f32, tag="b2")
            nc.sync.dma_start(out=b2_sb[:], in_=t_b2b[:, :])
            ident = cpool.tile([128, 128], f32, tag="ident")
            make_identity(nc, ident[:])

            # internal DRAM
            table1 = dram.tile([NP, W], f32, tag="table1")
            adst1 = dram.tile([NP, H], f32, tag="adst1")
            h2own = dram.tile([OWN_PAD, W], f32, tag="h2own")
            adst2 = dram.tile([OWN_PAD, H], f32, tag="adst2")
            table2 = dram.tile([NP, W], f32, tag="table2")

            # ---------------- phase A: build table1 (all node blocks) -----
            with (
                tc.tile_pool(name="pa", bufs=3) as pa,
                tc.tile_pool(name="pa_ps", bufs=2, space="PSUM") as pa_ps,
            ):
                for i in range(NB_ALL):
                    xt = pa.tile([128, KC * 128], f32, tag="xt")
                    nc.sync.dma_start(
                        out=fv(xt[:], [[128, KC], [1, 128]]),
                        in_=t_xTb[:, :, i * 128:(i + 1) * 128])
                    ph = pa_ps.tile([128, HC], f32, tag="ph")
                    for k in range(KC):
                        nc.tensor.matmul(
                            out=ph[:],
                            lhsT=xt[:, k * 128:(k + 1) * 128],
                            rhs=W1_sb[:, k * HC:(k + 1) * HC],
                            start=(k == 0), stop=(k == KC - 1))
                    hb = pa.tile([128, W], f32, tag="hb")
                    nc.scalar.copy(out=hb[:, :HC], in_=ph[:])
                    tmp = pa.tile([128, HC], f32, tag="tmp")
                    nc.vector.tensor_mul(out=tmp[:], in0=ph[:], in1=attS1_sb[:])
                    nc.vector.reduce_sum(
                        out=hb[:, HC:W], in_=fv(tmp[:], [[C, H], [1, C]]),
                        axis=X)
                    tmp2 = pa.tile([128, HC], f32, tag="tmp2")
                    nc.vector.tensor_mul(out=tmp2[:], in0=ph[:], in1=attD1_sb[:])
                    adt = pa.tile([128, H], f32, tag="adt")
                    nc.vector.reduce_sum(
                        out=adt[:], in_=fv(tmp2[:], [[C, H], [1, C]]), axis=X)
                    nc.sync.dma_start(
                        out=table1[i * 128:(i + 1) * 128, :], in_=hb[:])
                    nc.sync.dma_start(
                        out=adst1[i * 128:(i + 1) * 128, :], in_=adt[:])

            # ---------------- edge phase (shared between layers) ----------
            def edge_phase(tag, tab, adst_tab, t_didx, finish):
                with (
                    tc.tile_pool(name=f"eg{tag}", bufs=2) as eg,
                    tc.tile_pool(name=f"ew{tag}", bufs=2) as ew,
                    tc.tile_pool(name=f"ef{tag}", bufs=2) as ef,
                    tc.tile_pool(name=f"eps{tag}", bufs=2, space="PSUM") as eps,
                    tc.tile_pool(name=f"fps{tag}", bufs=2, space="PSUM") as fps,
                ):
                    for b in range(BLK):
                        T = Tb[b]
                        o = off[b]
                        idx = eg.tile([128, T], i32, tag="idx")
                        nc.sync.dma_start(out=idx[:], in_=t_srcp[:, o:o + T])
                        didx = eg.tile([128, T], i32, tag="didx")
                        nc.sync.dma_start(out=didx[:], in_=t_didx[:, o:o + T])
                        slot = eg.tile([128, T], f32, tag="slot")
                        nc.sync.dma_start(out=slot[:], in_=t_slot[:, o:o + T])

                        gath = eg.tile([128, T * W], f32, tag="gath")
                        nc.gpsimd.indirect_dma_start(
                            out=gath[:], out_offset=None,
                            in_=tab[:, :],
                            in_offset=bass.IndirectOffsetOnAxis(
                                ap=idx[:, :], axis=0))
                        gd = eg.tile([128, T * H], f32, tag="gd")
                        nc.gpsimd.indirect_dma_start(
                            out=gd[:], out_offset=None,
                            in_=adst_tab[:, :],
                            in_offset=bass.IndirectOffsetOnAxis(
                                ap=didx[:, :], axis=0))

                        # one-hot Pm[e, (j, d)] = (slot[e, j] == d)
                        Pm = ew.tile([128, T * 128], f32, tag="Pm")
                        nc.vector.tensor_tensor(
                            out=fv(Pm[:], [[128, T], [1, 128]]),
                            in0=fv(slot[:], [[1, T], [0, 128]]),
                            in1=fv(iota_sb[:], [[0, T], [1, 128]]),
                            op=mybir.AluOpType.is_equal)

                        # alpha = a_src[src] + a_dst[dst]; ex = exp(lrelu(alpha))
                        ax = ef.tile([128, T * H], f32, tag="ax")
                        nc.vector.tensor_add(
                            out=ax[:],
                            in0=fv(gath[:], [[W, T], [1, H]], extra_offset=HC),
                            in1=gd[:])
                        ax2 = ef.tile([128, T * H], f32, tag="ax2")
                        nc.scalar.mul(out=ax2[:], in_=ax[:], mul=NEG_SLOPE)
                        nc.vector.tensor_max(out=ax[:], in0=ax[:], in1=ax2[:])
                        ex = ef.tile([128, T * H], f32, tag="ex")
                        nc.scalar.activation(out=ex[:], in_=ax[:], func=AF.Exp)

                        # weighted features (+ ex appended per tile): [T, 136]
                        wt = ew.tile([128, T * (W)], f32, tag="wt")
                        nc.vector.tensor_mul(
                            out=fv(wt[:], [[W, T], [C, H], [1, C]]),
                            in0=fv(gath[:], [[W, T], [C, H], [1, C]]),
                            in1=fv(ex[:], [[H, T], [1, H], [0, C]]))
                        nc.scalar.copy(
                            out=fv(wt[:], [[W, T], [1, H]], extra_offset=HC),
                            in_=ex[:])

                        nd = eps.tile([128, W], f32, tag="nd")
                        for j in range(T):
                            nc.tensor.matmul(
                                out=nd[:],
                                lhsT=Pm[:, j * 128:(j + 1) * 128],
                                rhs=wt[:, j * W:(j + 1) * W],
                                start=(j == 0), stop=(j == T - 1))
                        finish(b, nd, ef, fps)

            # ---------------- layer-1 block finisher ----------------------
            def finish1(b, nd, ef, fps):
                dr = ef.tile([128, H], f32, tag="dr")
                nc.vector.tensor_scalar_add(dr[:], nd[:, HC:W], EPS)
                nc.vector.reciprocal(out=dr[:], in_=dr[:])
                g = ef.tile([128, HC], f32, tag="g")
                nc.vector.tensor_tensor(
                    out=fv(g[:], [[C, H], [1, C]]),
                    in0=fv(nd[:], [[C, H], [1, C]]),
                    in1=fv(dr[:], [[1, H], [0, C]]),
                    op=mybir.AluOpType.mult)
                nc.vector.tensor_add(out=g[:], in0=g[:], in1=b1_sb[:])
                # ELU
                tn = ef.tile([128, HC], f32, tag="tn")
                nc.vector.tensor_scalar_min(tn[:], g[:], 0.0)
                te = ef.tile([128, HC], f32, tag="te")
                nc.scalar.activation(out=te[:], in_=tn[:], func=AF.Exp)
                nc.vector.tensor_scalar(
                    out=g[:], in0=g[:], scalar1=0.0, scalar2=-1.0,
                    op0=mybir.AluOpType.max, op1=mybir.AluOpType.add)
                nc.vector.tensor_add(out=g[:], in0=g[:], in1=te[:])
                # a_src2 / a_dst2 via g @ (W2 @ att2): needs gT as lhsT
                gtp = fps.tile([128, 128], f32, tag="gtp")
                nc.tensor.transpose(out=gtp[:], in_=g[:], identity=ident[:])
                gts = ef.tile([128, 128], f32, tag="gts")
                nc.scalar.copy(out=gts[:], in_=gtp[:])
                a2p = fps.tile([128, 2 * H], f32, tag="a2p")
                nc.tensor.matmul(out=a2p[:], lhsT=gts[:], rhs=vavd_sb[:],
                                 start=True, stop=True)
                a2s = ef.tile([128, 2 * H], f32, tag="a2s")
                nc.scalar.copy(out=a2s[:], in_=a2p[:])
                nc.sync.dma_start(
                    out=h2own[b * 128:(b + 1) * 128, :HC], in_=g[:])
                nc.sync.dma_start(
                    out=h2own[b * 128:(b + 1) * 128, HC:W], in_=a2s[:, :H])
                nc.sync.dma_start(
                    out=adst2[b * 128:(b + 1) * 128, :], in_=a2s[:, H:])

            # ---------------- layer-2 block finisher ----------------------
            def finish2(b, nd, ef, fps):
                dr = ef.tile([128, H], f32, tag="dr")
                nc.vector.tensor_scalar_add(dr[:], nd[:, HC:W], EPS)
                nc.vector.reciprocal(out=dr[:], in_=dr[:])
                g = ef.tile([128, HC], f32, tag="g")
                nc.vector.tensor_tensor(
                    out=fv(g[:], [[C, H], [1, C]]),
                    in0=fv(nd[:], [[C, H], [1, C]]),
                    in1=fv(dr[:], [[1, H], [0, C]]),
                    op=mybir.AluOpType.mult)
                atp = fps.tile([128, 128], f32, tag="gtp")
                nc.tensor.transpose(out=atp[:], in_=g[:], identity=ident[:])
                ats = ef.tile([128, 128], f32, tag="gts")
                nc.scalar.copy(out=ats[:], in_=atp[:])
                o2 = fps.tile([128, HC], f32, tag="o2")
                nc.tensor.matmul(out=o2[:], lhsT=ats[:], rhs=W2_sb[:],
                                 start=True, stop=True)
                red = ef.tile([128, C], f32, tag="red")
                nc.vector.reduce_sum(
                    out=red[:], in_=fv(o2[:], [[1, C], [C, H]]), axis=X)
                nc.vector.tensor_scalar_mul(red[:], red[:], 1.0 / H)
                nc.vector.tensor_add(out=red[:], in0=red[:], in1=b2_sb[:])
                # log_softmax over the C classes
                mx = ef.tile([128, 1], f32, tag="mx")
                nc.vector.reduce_max(out=mx[:], in_=red[:], axis=X)
                nc.vector.tensor_sub(
                    out=red[:], in0=red[:], in1=mx[:].to_broadcast([128, C]))
                es = ef.tile([128, C], f32, tag="es")
                nc.scalar.activation(out=es[:], in_=red[:], func=AF.Exp)
                sm = ef.tile([128, 1], f32, tag="sm")
                nc.vector.reduce_sum(out=sm[:], in_=es[:], axis=X)
                ls = ef.tile([128, 1], f32, tag="ls")
                nc.scalar.activation(out=ls[:], in_=sm[:], func=AF.Ln)
                nc.vector.tensor_sub(
                    out=red[:], in0=red[:], in1=ls[:].to_broadcast([128, C]))
                nc.sync.dma_start(
                    out=t_out[b * 128:(b + 1) * 128, :], in_=red[:])

            # ---------------- run both layers ------------------------------
            edge_phase("1", table1, adst1, t_dstp, finish1)

            nc.gpsimd.collective_compute(
                "AllGather",
                mybir.AluOpType.bypass,
                replica_groups=[list(range(NC))],
                ins=[h2own[:].opt()],
                outs=[table2[:].opt()],
            )

            edge_phase("2", table2, adst2, t_dstl, finish2)

    nc.compile()
    return nc


# ---------------------------------------------------------------------------
# entry point
# ---------------------------------------------------------------------------

def _run(cfg, inputs, trace=False):
    from concourse.bass_utils import run_bass_kernel_spmd

    cfg = _derive(cfg)
    in_maps, Tb, off, totT = _host_prep(cfg, **inputs)
    nc = build_program(cfg, Tb, off, totT)
    res = run_bass_kernel_spmd(
        nc, in_maps, core_ids=list(range(cfg["NC"])), trace=trace)
    outs = []
    for m in range(cfg["NC"]):
        outs.append(res.results[m]["out"][:cfg["OWN"]])
    full = np.concatenate(outs, axis=0)
    return full, res


def kernel(x, edge_index, W1, att_src1, att_dst1, bias1, W2, att_src2,
           att_dst2, bias2):
    inputs = dict(x=np.asarray(x, np.float32),
                  edge_index=np.asarray(edge_index),
                  W1=W1, att_src1=att_src1, att_dst1=att_dst1, bias1=bias1,
                  W2=W2, att_src2=att_src2, att_dst2=att_dst2, bias2=bias2)
    out, _ = _run(FULL_CFG, inputs, trace=False)
    return out



# revision 4
# speedup vs baseline: 1.3571x; 1.3571x over previous
"""Two-layer GAT on 8 Trainium2 NeuronCores (Bass/Tile SPMD kernel).

Sharding: nodes are range-partitioned across the 8 cores; each core owns the
edges whose *destination* falls in its partition (segment softmax is per-dst,
so the softmax/aggregation is fully core-local). Each core computes the
feature table only for its own node shard; the full table is then
AllGathered (fp16, Shared-output HBM collective) before each edge phase.

Per-edge math uses the shift-invariance of softmax:
    out_d = sum_e exp(alpha_e) * h[src_e] / sum_e exp(alpha_e)
so a single pass over the edges suffices (alpha is O(1) here so exp cannot
overflow).  The per-edge scatter-add onto destination nodes is a one-hot
matmul: edges are grouped host-side by (dst block, 32-wide dst group) so the
one-hot matrix is only [128, 32] per tile.

a_src / a_dst projections are folded into the feature matmul by extending the
weight matrix with host-precomputed columns  u[k, h] = sum_c W[k, h*C+c] *
att[h, c], so the per-node phase is a single matmul per 128-node block.
"""

import sys

sys.path.insert(0, "/opt/trn_rl_repo")

import numpy as np

# ---------------------------------------------------------------------------
# configuration
# ---------------------------------------------------------------------------

FULL_CFG = dict(
    N=100000,      # real nodes
    IN_CH=512,     # input features
    HEADS=8,
    C=16,          # out channels per head
    NC=8,          # cores
)

NEG_SLOPE = 0.2
EPS = 1e-16
GRP = 64           # dst-group width for the one-hot scatter
PAD_SLOT = 999.0


def _derive(cfg):
    d = dict(cfg)
    d["HC"] = d["HEADS"] * d["C"]                 # 128
    assert d["HC"] == 128
    assert d["IN_CH"] % 128 == 0
    d["KC"] = d["IN_CH"] // 128                   # k-chunks for x@W1
    assert d["N"] % d["NC"] == 0
    d["OWN"] = d["N"] // d["NC"]                  # real nodes per core
    d["BLK"] = (d["OWN"] + 127) // 128            # dst blocks per core
    d["OWN_PAD"] = d["BLK"] * 128
    d["NP"] = d["NC"] * d["OWN_PAD"]              # padded global nodes
    d["NG"] = 128 // GRP                          # groups per block
    return d


# ---------------------------------------------------------------------------
# host-side prep: edge partitioning / padding / layouts
# ---------------------------------------------------------------------------

def _host_prep(cfg, x, edge_index, W1, att_src1, att_dst1, bias1, W2,
               att_src2, att_dst2, bias2):
    N, NC, OWN, BLK, OWN_PAD, NP, KC, H, C, NG = (
        cfg["N"], cfg["NC"], cfg["OWN"], cfg["BLK"], cfg["OWN_PAD"],
        cfg["NP"], cfg["KC"], cfg["HEADS"], cfg["C"], cfg["NG"])
    HC = H * C
    NSEG = BLK * NG                       # scatter segments per core

    src = np.asarray(edge_index[0], dtype=np.int64)
    dst = np.asarray(edge_index[1], dtype=np.int64)

    core = dst // OWN                       # owning core of each edge
    ldst = (dst - core * OWN).astype(np.int64)    # local dst id [0, OWN)
    srcp = ((src // OWN) * OWN_PAD + (src % OWN)).astype(np.int32)  # padded gid
    seg = ldst // GRP                       # scatter segment [0, NSEG)
    slot = ldst % GRP

    # per-(core, segment) edge counts -> shared tile counts T_s (SPMD uniform)
    counts = np.zeros((NC, NSEG), dtype=np.int64)
    np.add.at(counts, (core, seg), 1)
    Ts = np.maximum(1, (counts.max(axis=0) + 127) // 128).astype(np.int64)
    off = np.zeros(NSEG, dtype=np.int64)
    off[1:] = np.cumsum(Ts)[:-1]
    totT = int(Ts.sum())

    # per-core edge arrays, laid out [128, totT]:
    #   edge e of segment s  ->  (partition p = e%128, column off[s] + e//128)
    srcp_a = np.zeros((NC, 128, totT), dtype=np.int32)
    dstl_a = np.zeros((NC, 128, totT), dtype=np.int32)
    slot_a = np.full((NC, 128, totT), PAD_SLOT, dtype=np.float16)

    order = np.lexsort((seg, core))
    s_core, s_seg = core[order], seg[order]
    s_srcp, s_ldst, s_slot = srcp[order], ldst[order], slot[order]
    grp = s_core * NSEG + s_seg
    first = np.ones(len(grp), dtype=bool)
    first[1:] = grp[1:] != grp[:-1]
    starts = np.flatnonzero(first)
    group_start = np.repeat(starts, np.diff(np.append(starts, len(grp))))
    pos = np.arange(len(grp)) - group_start
    pp = pos % 128
    col = off[s_seg] + pos // 128
    srcp_a[s_core, pp, col] = s_srcp
    dstl_a[s_core, pp, col] = s_ldst
    slot_a[s_core, pp, col] = s_slot.astype(np.float16)

    # per-core x shard, fp16, transposed-contiguous for matmul lhsT:
    # xTc[p, ((i*KC + k)*128) + g] = x[own(i*128+g), k*128 + p]
    xTc_list = []
    for m in range(NC):
        xpad = np.zeros((OWN_PAD, cfg["IN_CH"]), dtype=np.float32)
        xpad[:OWN] = x[m * OWN:(m + 1) * OWN]
        t = xpad.reshape(BLK, 128, KC, 128).transpose(3, 0, 2, 1)
        xTc_list.append(np.ascontiguousarray(
            t.reshape(128, BLK * KC * 128).astype(np.float16)))

    # W1 extended with a_src / a_dst projection columns, [128, KC*(HC+2H)]
    W1f = np.asarray(W1, np.float32)
    u_src1 = np.einsum("khc,hc->kh", W1f.reshape(cfg["IN_CH"], H, C),
                       np.asarray(att_src1, np.float32))
    u_dst1 = np.einsum("khc,hc->kh", W1f.reshape(cfg["IN_CH"], H, C),
                       np.asarray(att_dst1, np.float32))
    WA = HC + 2 * H                                   # 144
    W1e = np.concatenate([W1f, u_src1, u_dst1], axis=1)   # [IN_CH, 144]
    W1ex = np.ascontiguousarray(
        W1e.reshape(KC, 128, WA).transpose(1, 0, 2)
        .reshape(128, KC * WA).astype(np.float16))

    iota = np.broadcast_to(
        np.arange(128, dtype=np.float16)[None, :], (128, 128)).copy()
    b1b = np.broadcast_to(
        np.asarray(bias1, np.float32).reshape(1, HC), (128, HC)).copy()
    W2f = np.asarray(W2, np.float32)
    va = np.einsum("khc,hc->kh", W2f.reshape(HC, H, C),
                   np.asarray(att_src2, np.float32))
    vd = np.einsum("khc,hc->kh", W2f.reshape(HC, H, C),
                   np.asarray(att_dst2, np.float32))
    vavd = np.ascontiguousarray(
        np.concatenate([va, vd], axis=1).astype(np.float16))  # [128, 2H]
    W2h = np.ascontiguousarray(W2f.astype(np.float16))
    b2b = np.broadcast_to(
        np.asarray(bias2, np.float32).reshape(1, C), (128, C)).copy()

    shared = dict(xTc=None, W1ex=W1ex, iota=iota, b1b=b1b, W2=W2h,
                  vavd=vavd, b2b=b2b)
    in_maps = []
    for m in range(NC):
        im = dict(shared)
        im["xTc"] = xTc_list[m]
        im["srcp_a"] = np.ascontiguousarray(srcp_a[m])
        im["dstl_a"] = np.ascontiguousarray(dstl_a[m])
        im["slot_a"] = np.ascontiguousarray(slot_a[m])
        in_maps.append(im)

    return in_maps, Ts.tolist(), off.tolist(), totT


# ---------------------------------------------------------------------------
# device program
# ---------------------------------------------------------------------------

def build_program(cfg, Ts, off, totT):
    from concourse import bacc, bass, mybir, tile
    from concourse.masks import make_identity

    f32 = mybir.dt.float32
    f16 = mybir.dt.float16
    i32 = mybir.dt.int32
    X = mybir.AxisListType.X
    AF = mybir.ActivationFunctionType
    NC, NP, OWN_PAD, BLK, KC, H, C, NG = (
        cfg["NC"], cfg["NP"], cfg["OWN_PAD"], cfg["BLK"], cfg["KC"],
        cfg["HEADS"], cfg["C"], cfg["NG"])
    HC = H * C
    W = 8 + HC               # table row width: [h (128) | a_src (8)]
    WA = HC + 2 * H          # phase-A psum width (h | a_src | a_dst)

    nc = bacc.Bacc("TRN2", target_bir_lowering=False, debug=False,
                   num_devices=NC)

    # inputs
    t_xTc = nc.dram_tensor("xTc", [128, BLK * KC * 128], f16,
                           kind="ExternalInput")
    t_W1ex = nc.dram_tensor("W1ex", [128, KC * WA], f16, kind="ExternalInput")
    t_iota = nc.dram_tensor("iota", [128, 128], f16, kind="ExternalInput")
    t_b1b = nc.dram_tensor("b1b", [128, HC], f32, kind="ExternalInput")
    t_W2 = nc.dram_tensor("W2", [HC, HC], f16, kind="ExternalInput")
    t_vavd = nc.dram_tensor("vavd", [HC, 2 * H], f16, kind="ExternalInput")
    t_b2b = nc.dram_tensor("b2b", [128, C], f32, kind="ExternalInput")
    t_srcp = nc.dram_tensor("srcp_a", [128, totT], i32, kind="ExternalInput")
    t_dstl = nc.dram_tensor("dstl_a", [128, totT], i32, kind="ExternalInput")
    t_slot = nc.dram_tensor("slot_a", [128, totT], f16, kind="ExternalInput")
    t_out = nc.dram_tensor("out", [OWN_PAD, C], f32, kind="ExternalOutput")

    def fv(ap, dims, extra_offset=0):
        """View `ap` with custom free-dim [step, count] pairs."""
        return bass.AP(ap.tensor, ap.offset + extra_offset, [ap.ap[0]] + dims)

    with tile.TileContext(nc) as tc:
        with (
            tc.tile_pool(name="const", bufs=1) as cpool,
            tc.tile_pool(name="dram", bufs=1, space="DRAM") as dram,
        ):
            # ---------------- constants ----------------
            W1_sb = cpool.tile([128, KC * WA], f16, tag="w1")
            nc.sync.dma_start(out=W1_sb[:], in_=t_W1ex[:, :])
            iota_sb = cpool.tile([128, 128], f16, tag="iota")
            nc.sync.dma_start(out=iota_sb[:], in_=t_iota[:, :])
            b1_sb = cpool.tile([128, HC], f32, tag="b1")
            nc.sync.dma_start(out=b1_sb[:], in_=t_b1b[:, :])
            W2_sb = cpool.tile([HC, HC], f16, tag="w2")
            nc.sync.dma_start(out=W2_sb[:], in_=t_W2[:, :])
            vavd_sb = cpool.tile([HC, 2 * H], f16, tag="vavd")
            nc.sync.dma_start(out=vavd_sb[:], in_=t_vavd[:, :])
            b2_sb = cpool.tile([128, C], f32, tag="b2")
            nc.sync.dma_start(out=b2_sb[:], in_=t_b2b[:, :])
            ident = cpool.tile([128, 128], f16, tag="ident")
            make_identity(nc, ident[:])

            # internal DRAM
            h1own = dram.tile([OWN_PAD, W], f16, tag="h1own")
            adst1 = dram.tile([OWN_PAD, H], f16, tag="adst1")
            table1 = dram.tile([NP, W], f16, tag="table1",
                               addr_space="Shared")
            h2own = dram.tile([OWN_PAD, W], f16, tag="h2own")
            adst2 = dram.tile([OWN_PAD, H], f16, tag="adst2")
            table2 = dram.tile([NP, W], f16, tag="table2",
                               addr_space="Shared")

            # ---------------- phase A: own-shard table1 -------------------
            with (
                tc.tile_pool(name="pa", bufs=3) as pa,
                tc.tile_pool(name="pa_ps", bufs=2, space="PSUM") as pa_ps,
            ):
                for i in range(BLK):
                    xt = pa.tile([128, KC * 128], f16, tag="xt")
                    nc.sync.dma_start(
                        out=xt[:],
                        in_=t_xTc[:, i * KC * 128:(i + 1) * KC * 128])
                    ph = pa_ps.tile([128, WA], f32, tag="ph")
                    for k in range(KC):
                        nc.tensor.matmul(
                            out=ph[:],
                            lhsT=xt[:, k * 128:(k + 1) * 128],
                            rhs=W1_sb[:, k * WA:(k + 1) * WA],
                            start=(k == 0), stop=(k == KC - 1))
                    hb = pa.tile([128, WA], f16, tag="hb")
                    nc.scalar.copy(out=hb[:], in_=ph[:])
                    nc.sync.dma_start(
                        out=h1own[i * 128:(i + 1) * 128, :], in_=hb[:, :W])
                    nc.sync.dma_start(
                        out=adst1[i * 128:(i + 1) * 128, :],
                        in_=hb[:, W:WA])

            nc.gpsimd.collective_compute(
                "AllGather",
                mybir.AluOpType.bypass,
                replica_groups=[list(range(NC))],
                ins=[h1own[:].opt()],
                outs=[table1[:].opt()],
            )

            # ---------------- edge phase (shared between layers) ----------
            def edge_phase(tag, tab, adst_tab, finish):
                with (
                    tc.tile_pool(name=f"eg{tag}", bufs=2) as eg,
                    tc.tile_pool(name=f"ew{tag}", bufs=2) as ew,
                    tc.tile_pool(name=f"ef{tag}", bufs=2) as ef,
                    tc.tile_pool(name=f"eps{tag}", bufs=2, space="PSUM") as eps,
                    tc.tile_pool(name=f"fps{tag}", bufs=2, space="PSUM") as fps,
                ):
                    for b in range(BLK):
                        nd = eps.tile([128, W], f32, tag="nd")
                        for g in range(NG):
                            s = b * NG + g
                            T = Ts[s]
                            o = off[s]
                            idx = eg.tile([128, T], i32, tag="idx")
                            nc.sync.dma_start(
                                out=idx[:], in_=t_srcp[:, o:o + T])
                            didx = eg.tile([128, T], i32, tag="didx")
                            nc.sync.dma_start(
                                out=didx[:], in_=t_dstl[:, o:o + T])
                            slot = eg.tile([128, T], f16, tag="slot")
                            nc.sync.dma_start(
                                out=slot[:], in_=t_slot[:, o:o + T])

                            gath = eg.tile([128, T * W], f16, tag="gath")
                            nc.gpsimd.indirect_dma_start(
                                out=gath[:], out_offset=None,
                                in_=tab[:, :],
                                in_offset=bass.IndirectOffsetOnAxis(
                                    ap=idx[:, :], axis=0))
                            gd = eg.tile([128, T * H], f16, tag="gd")
                            nc.gpsimd.indirect_dma_start(
                                out=gd[:], out_offset=None,
                                in_=adst_tab[:, :],
                                in_offset=bass.IndirectOffsetOnAxis(
                                    ap=didx[:, :], axis=0))

                            # one-hot Pm[e, (j, d)] = (slot[e, j] == d)
                            Pm = ew.tile([128, T * GRP], f16, tag="Pm")
                            nc.vector.tensor_tensor(
                                out=fv(Pm[:], [[GRP, T], [1, GRP]]),
                                in0=fv(slot[:], [[1, T], [0, GRP]]),
                                in1=fv(iota_sb[:], [[0, T], [1, GRP]]),
                                op=mybir.AluOpType.is_equal)

                            # alpha = a_src[src] + a_dst[dst]
                            ax = ef.tile([128, T * H], f16, tag="ax")
                            nc.vector.tensor_add(
                                out=ax[:],
                                in0=fv(gath[:], [[W, T], [1, H]],
                                       extra_offset=HC),
                                in1=gd[:])
                            lr = ef.tile([128, T * H], f16, tag="lr")
                            nc.scalar.activation(
                                out=lr[:], in_=ax[:], func=AF.Lrelu,
                                alpha=NEG_SLOPE)

                            # wt = [h * ex | ex] per tile: [T, 136]
                            wt = ew.tile([128, T * W], f16, tag="wt")
                            nc.scalar.activation(
                                out=fv(wt[:], [[W, T], [1, H]],
                                       extra_offset=HC),
                                in_=lr[:], func=AF.Exp)
                            exb = ef.tile([128, T * HC], f16, tag="exb")
                            nc.scalar.activation(
                                out=fv(exb[:], [[HC, T], [C, H], [1, C]]),
                                in_=fv(lr[:], [[H, T], [1, H], [0, C]]),
                                func=AF.Exp)
                            nc.vector.tensor_mul(
                                out=fv(wt[:], [[W, T], [1, HC]]),
                                in0=fv(gath[:], [[W, T], [1, HC]]),
                                in1=exb[:])

                            for j in range(T):
                                nc.tensor.matmul(
                                    out=nd[g * GRP:(g + 1) * GRP, :],
                                    lhsT=Pm[:, j * GRP:(j + 1) * GRP],
                                    rhs=wt[:, j * W:(j + 1) * W],
                                    start=(j == 0), stop=(j == T - 1))
                        finish(b, nd, ef, fps)

            # ---------------- layer-1 block finisher ----------------------
            def finish1(b, nd, ef, fps):
                dr = ef.tile([128, H], f32, tag="dr")
                nc.vector.tensor_scalar_add(dr[:], nd[:, HC:W], EPS)
                nc.vector.reciprocal(out=dr[:], in_=dr[:])
                g = ef.tile([128, HC], f32, tag="g")
                nc.vector.tensor_tensor(
                    out=fv(g[:], [[C, H], [1, C]]),
                    in0=fv(nd[:], [[C, H], [1, C]]),
                    in1=fv(dr[:], [[1, H], [0, C]]),
                    op=mybir.AluOpType.mult)
                nc.vector.tensor_add(out=g[:], in0=g[:], in1=b1_sb[:])
                # ELU
                tn = ef.tile([128, HC], f32, tag="tn")
                nc.vector.tensor_scalar_min(tn[:], g[:], 0.0)
                te = ef.tile([128, HC], f32, tag="te")
                nc.scalar.activation(out=te[:], in_=tn[:], func=AF.Exp)
                nc.vector.tensor_scalar(
                    out=g[:], in0=g[:], scalar1=0.0, scalar2=-1.0,
                    op0=mybir.AluOpType.max, op1=mybir.AluOpType.add)
                gh = ef.tile([128, HC], f16, tag="gh")
                nc.vector.tensor_add(out=gh[:], in0=g[:], in1=te[:])
                # a_src2 / a_dst2 via g @ (W2 @ att2): needs gT as lhsT
                gtp = fps.tile([128, 128], f16, tag="gtp")
                nc.tensor.transpose(out=gtp[:], in_=gh[:], identity=ident[:])
                gts = ef.tile([128, 128], f16, tag="gts")
                nc.scalar.copy(out=gts[:], in_=gtp[:])
                a2p = fps.tile([128, 2 * H], f32, tag="a2p")
                nc.tensor.matmul(out=a2p[:], lhsT=gts[:], rhs=vavd_sb[:],
                                 start=True, stop=True)
                a2s = ef.tile([128, 2 * H], f16, tag="a2s")
                nc.scalar.copy(out=a2s[:], in_=a2p[:])
                nc.sync.dma_start(
                    out=h2own[b * 128:(b + 1) * 128, :HC], in_=gh[:])
                nc.sync.dma_start(
                    out=h2own[b * 128:(b + 1) * 128, HC:W], in_=a2s[:, :H])
                nc.sync.dma_start(
                    out=adst2[b * 128:(b + 1) * 128, :], in_=a2s[:, H:])

            # ---------------- layer-2 block finisher ----------------------
            def finish2(b, nd, ef, fps):
                dr = ef.tile([128, H], f32, tag="dr")
                nc.vector.tensor_scalar_add(dr[:], nd[:, HC:W], EPS)
                nc.vector.reciprocal(out=dr[:], in_=dr[:])
                g = ef.tile([128, HC], f16, tag="g")
                nc.vector.tensor_tensor(
                    out=fv(g[:], [[C, H], [1, C]]),
                    in0=fv(nd[:], [[C, H], [1, C]]),
                    in1=fv(dr[:], [[1, H], [0, C]]),
                    op=mybir.AluOpType.mult)
                atp = fps.tile([128, 128], f16, tag="gtp")
                nc.tensor.transpose(out=atp[:], in_=g[:], identity=ident[:])
                ats = ef.tile([128, 128], f16, tag="gts")
                nc.scalar.copy(out=ats[:], in_=atp[:])
                o2 = fps.tile([128, HC], f32, tag="o2")
                nc.tensor.matmul(out=o2[:], lhsT=ats[:], rhs=W2_sb[:],
                                 start=True, stop=True)
                red = ef.tile([128, C], f32, tag="red")
                nc.vector.reduce_sum(
                    out=red[:], in_=fv(o2[:], [[1, C], [C, H]]), axis=X)
                nc.vector.tensor_scalar_mul(red[:], red[:], 1.0 / H)
                nc.vector.tensor_add(out=red[:], in0=red[:], in1=b2_sb[:])
                # log_softmax over the C classes
                mx = ef.tile([128, 1], f32, tag="mx")
                nc.vector.reduce_max(out=mx[:], in_=red[:], axis=X)
                nc.vector.tensor_sub(
                    out=red[:], in0=red[:], in1=mx[:].to_broadcast([128, C]))
                es = ef.tile([128, C], f32, tag="es")
                nc.scalar.activation(out=es[:], in_=red[:], func=AF.Exp)
                sm = ef.tile([128, 1], f32, tag="sm")
                nc.vector.reduce_sum(out=sm[:], in_=es[:], axis=X)
                ls = ef.tile([128, 1], f32, tag="ls")
                nc.scalar.activation(out=ls[:], in_=sm[:], func=AF.Ln)
                nc.vector.tensor_sub(
                    out=red[:], in0=red[:], in1=ls[:].to_broadcast([128, C]))
                nc.sync.dma_start(
                    out=t_out[b * 128:(b + 1) * 128, :], in_=red[:])

            # ---------------- run both layers ------------------------------
            edge_phase("1", table1, adst1, finish1)

            nc.gpsimd.collective_compute(
                "AllGather",
                mybir.AluOpType.bypass,
                replica_groups=[list(range(NC))],
                ins=[h2own[:].opt()],
                outs=[table2[:].opt()],
            )

            edge_phase("2", table2, adst2, finish2)

    nc.compile()
    return nc


# ---------------------------------------------------------------------------
# entry point
# ---------------------------------------------------------------------------

def _run(cfg, inputs, trace=False):
    from concourse.bass_utils import run_bass_kernel_spmd

    cfg = _derive(cfg)
    in_maps, Ts, off, totT = _host_prep(cfg, **inputs)
    nc = build_program(cfg, Ts, off, totT)
    res = run_bass_kernel_spmd(
        nc, in_maps, core_ids=list(range(cfg["NC"])), trace=trace)
    outs = []
    for m in range(cfg["NC"]):
        outs.append(res.results[m]["out"][:cfg["OWN"]])
    full = np.concatenate(outs, axis=0)
    return full, res


def kernel(x, edge_index, W1, att_src1, att_dst1, bias1, W2, att_src2,
           att_dst2, bias2):
    inputs = dict(x=np.asarray(x, np.float32),
                  edge_index=np.asarray(edge_index),
                  W1=W1, att_src1=att_src1, att_dst1=att_dst1, bias1=bias1,
                  W2=W2, att_src2=att_src2, att_dst2=att_dst2, bias2=bias2)
    out, _ = _run(FULL_CFG, inputs, trace=False)
    return out


# revision 10
# speedup vs baseline: 1.8797x; 1.3851x over previous
"""Two-layer GAT on 8 Trainium2 NeuronCores (Bass/Tile SPMD kernel).

Sharding: nodes are range-partitioned across the 8 cores; each core owns the
edges whose *destination* falls in its partition (segment softmax is per-dst,
so softmax/aggregation is core-local). Each core computes the feature table
only for its own shard; the full table is AllGathered (fp16, Shared-output
HBM collective, split in two halves so the first half overlaps compute).

Per-edge math uses the shift-invariance of softmax:
    out_d = sum_e exp(alpha_e) * h[src_e] / sum_e exp(alpha_e)
(alpha is O(1) here so exp cannot overflow).  The per-edge scatter-add onto
destinations is a one-hot matmul; edges are grouped host-side by 64-wide dst
group so the one-hot is [128, 64] per tile (PSUM base partitions 0/64 only).

Feature tables use a c-major layout (col j = c*HEADS + h) so the per-edge
attention coefficient can be broadcast across channels with a contiguous
step-1 fp16 copy (fast DVE mode) instead of a strided one.

a_src/a_dst projections are folded into the feature matmul via
host-precomputed extension columns u[k, h] = sum_c W[k, hc] * att[h, c].
"""

import sys

sys.path.insert(0, "/opt/trn_rl_repo")

import numpy as np

# ---------------------------------------------------------------------------
# configuration
# ---------------------------------------------------------------------------

FULL_CFG = dict(
    N=100000,      # real nodes
    IN_CH=512,     # input features
    HEADS=8,
    C=16,          # out channels per head
    NC=8,          # cores
)

NEG_SLOPE = 0.2
EPS = 1e-16
GRP = 64           # dst-group width for the one-hot scatter
SB = 2             # dst blocks batched per gather iteration
HALVES = 1         # AllGather split (Shared tables allow only one writer)
PAD_SLOT = 999.0


def _derive(cfg):
    d = dict(cfg)
    d["HC"] = d["HEADS"] * d["C"]                 # 128
    assert d["HC"] == 128
    assert d["IN_CH"] % 128 == 0
    d["KC"] = d["IN_CH"] // 128                   # k-chunks for x@W1
    assert d["N"] % d["NC"] == 0
    d["OWN"] = d["N"] // d["NC"]                  # real nodes per core
    d["BLK"] = (d["OWN"] + 127) // 128            # dst blocks per core
    d["OWN_PAD"] = d["BLK"] * 128
    d["NP"] = d["NC"] * d["OWN_PAD"]              # padded global nodes
    d["NG"] = 128 // GRP                          # groups per block
    assert d["BLK"] % SB == 0 and d["BLK"] % HALVES == 0
    return d


# ---------------------------------------------------------------------------
# host-side prep: edge partitioning / padding / layouts
# ---------------------------------------------------------------------------

def _host_prep(cfg, x, edge_index, W1, att_src1, att_dst1, bias1, W2,
               att_src2, att_dst2, bias2):
    N, NC, OWN, BLK, OWN_PAD, NP, KC, H, C, NG = (
        cfg["N"], cfg["NC"], cfg["OWN"], cfg["BLK"], cfg["OWN_PAD"],
        cfg["NP"], cfg["KC"], cfg["HEADS"], cfg["C"], cfg["NG"])
    HC = H * C
    NSEG = BLK * NG                       # scatter segments per core
    ROWS_H = OWN_PAD // HALVES

    src = np.asarray(edge_index[0], dtype=np.int64)
    dst = np.asarray(edge_index[1], dtype=np.int64)

    core = dst // OWN                       # owning core of each edge
    ldst = (dst - core * OWN).astype(np.int64)    # local dst id [0, OWN)
    # table row of a src node under the half-concat AllGather layout
    score = src // OWN
    slocal = src % OWN
    shalf = (slocal >= ROWS_H).astype(np.int64)
    srcp = (shalf * NC * ROWS_H + score * ROWS_H
            + (slocal - shalf * ROWS_H)).astype(np.int32)
    seg = ldst // GRP                       # scatter segment [0, NSEG)
    slot = ldst % GRP

    # per-(core, segment) edge counts -> shared tile counts T_s (SPMD uniform)
    counts = np.zeros((NC, NSEG), dtype=np.int64)
    np.add.at(counts, (core, seg), 1)
    Ts = np.maximum(1, (counts.max(axis=0) + 127) // 128).astype(np.int64)
    off = np.zeros(NSEG, dtype=np.int64)
    off[1:] = np.cumsum(Ts)[:-1]
    totT = int(Ts.sum())

    # per-core edge arrays, laid out [128, totT]:
    #   edge e of segment s  ->  (partition p = e%128, column off[s] + e//128)
    srcp_a = np.zeros((NC, 128, totT), dtype=np.int32)
    dstl_a = np.zeros((NC, 128, totT), dtype=np.int32)
    slot_a = np.full((NC, 128, totT), PAD_SLOT, dtype=np.float16)

    order = np.lexsort((seg, core))
    s_core, s_seg = core[order], seg[order]
    s_srcp, s_ldst, s_slot = srcp[order], ldst[order], slot[order]
    grp = s_core * NSEG + s_seg
    first = np.ones(len(grp), dtype=bool)
    first[1:] = grp[1:] != grp[:-1]
    starts = np.flatnonzero(first)
    group_start = np.repeat(starts, np.diff(np.append(starts, len(grp))))
    pos = np.arange(len(grp)) - group_start
    pp = pos % 128
    col = off[s_seg] + pos // 128
    srcp_a[s_core, pp, col] = s_srcp
    dstl_a[s_core, pp, col] = s_ldst
    slot_a[s_core, pp, col] = s_slot.astype(np.float16)

    # per-core x shard, fp16, transposed-contiguous for matmul lhsT:
    # xTc[p, ((i*KC + k)*128) + g] = x[own(i*128+g), k*128 + p]
    xTc_list = []
    for m in range(NC):
        xpad = np.zeros((OWN_PAD, cfg["IN_CH"]), dtype=np.float32)
        xpad[:OWN] = x[m * OWN:(m + 1) * OWN]
        t = xpad.reshape(BLK, 128, KC, 128).transpose(3, 0, 2, 1)
        xTc_list.append(np.ascontiguousarray(
            t.reshape(128, BLK * KC * 128).astype(np.float16)))

    # c-major feature permutation: table col j = c*H + h  <->  orig h*C + c
    jj = np.arange(HC)
    permJ = (jj % H) * C + jj // H

    # W1 extended with a_src / a_dst projection columns, [128, KC*(HC+2H)]
    W1f = np.asarray(W1, np.float32)
    u_src1 = np.einsum("khc,hc->kh", W1f.reshape(cfg["IN_CH"], H, C),
                       np.asarray(att_src1, np.float32))
    u_dst1 = np.einsum("khc,hc->kh", W1f.reshape(cfg["IN_CH"], H, C),
                       np.asarray(att_dst1, np.float32))
    WA = HC + 2 * H                                   # 144
    W1e = np.concatenate([W1f[:, permJ], u_src1, u_dst1], axis=1)
    W1ex = np.ascontiguousarray(
        W1e.reshape(KC, 128, WA).transpose(1, 0, 2)
        .reshape(128, KC * WA).astype(np.float16))

    iota = np.broadcast_to(
        np.arange(128, dtype=np.float16)[None, :], (128, 128)).copy()
    b1b = np.broadcast_to(
        np.asarray(bias1, np.float32)[permJ].reshape(1, HC), (128, HC)).copy()
    W2f = np.asarray(W2, np.float32)
    va = np.einsum("khc,hc->kh", W2f.reshape(HC, H, C),
                   np.asarray(att_src2, np.float32))
    vd = np.einsum("khc,hc->kh", W2f.reshape(HC, H, C),
                   np.asarray(att_dst2, np.float32))
    vavd = np.ascontiguousarray(
        np.concatenate([va, vd], axis=1)[permJ, :].astype(np.float16))
    W2h = np.ascontiguousarray(W2f[permJ, :].astype(np.float16))
    b2b = np.broadcast_to(
        np.asarray(bias2, np.float32).reshape(1, C), (128, C)).copy()

    shared = dict(xTc=None, W1ex=W1ex, iota=iota, b1b=b1b, W2=W2h,
                  vavd=vavd, b2b=b2b)
    in_maps = []
    for m in range(NC):
        im = dict(shared)
        im["xTc"] = xTc_list[m]
        im["srcp_a"] = np.ascontiguousarray(srcp_a[m])
        im["dstl_a"] = np.ascontiguousarray(dstl_a[m])
        im["slot_a"] = np.ascontiguousarray(slot_a[m])
        in_maps.append(im)

    return in_maps, Ts.tolist(), off.tolist(), totT


# ---------------------------------------------------------------------------
# device program
# ---------------------------------------------------------------------------

def build_program(cfg, Ts, off, totT):
    from concourse import bacc, bass, mybir, tile
    from concourse.masks import make_identity

    f32 = mybir.dt.float32
    f16 = mybir.dt.float16
    i32 = mybir.dt.int32
    X = mybir.AxisListType.X
    AF = mybir.ActivationFunctionType
    NC, NP, OWN_PAD, BLK, KC, H, C, NG = (
        cfg["NC"], cfg["NP"], cfg["OWN_PAD"], cfg["BLK"], cfg["KC"],
        cfg["HEADS"], cfg["C"], cfg["NG"])
    HC = H * C
    W = 8 + HC               # table row width: [h (128, c-major) | a_src (8)]
    WA = HC + 2 * H          # phase-A psum width (h | a_src | a_dst)
    ROWS_H = OWN_PAD // HALVES

    nc = bacc.Bacc("TRN2", target_bir_lowering=False, debug=False,
                   num_devices=NC)

    # inputs
    t_xTc = nc.dram_tensor("xTc", [128, BLK * KC * 128], f16,
                           kind="ExternalInput")
    t_W1ex = nc.dram_tensor("W1ex", [128, KC * WA], f16, kind="ExternalInput")
    t_iota = nc.dram_tensor("iota", [128, 128], f16, kind="ExternalInput")
    t_b1b = nc.dram_tensor("b1b", [128, HC], f32, kind="ExternalInput")
    t_W2 = nc.dram_tensor("W2", [HC, HC], f16, kind="ExternalInput")
    t_vavd = nc.dram_tensor("vavd", [HC, 2 * H], f16, kind="ExternalInput")
    t_b2b = nc.dram_tensor("b2b", [128, C], f32, kind="ExternalInput")
    t_srcp = nc.dram_tensor("srcp_a", [128, totT], i32, kind="ExternalInput")
    t_dstl = nc.dram_tensor("dstl_a", [128, totT], i32, kind="ExternalInput")
    t_slot = nc.dram_tensor("slot_a", [128, totT], f16, kind="ExternalInput")
    t_out = nc.dram_tensor("out", [OWN_PAD, C], f32, kind="ExternalOutput")

    def fv(ap, dims, extra_offset=0):
        """View `ap` with custom free-dim [step, count] pairs."""
        return bass.AP(ap.tensor, ap.offset + extra_offset, [ap.ap[0]] + dims)

    with tile.TileContext(nc) as tc:
        with (
            tc.tile_pool(name="const", bufs=1) as cpool,
            tc.tile_pool(name="dram", bufs=1, space="DRAM") as dram,
        ):
            # ---------------- constants ----------------
            W1_sb = cpool.tile([128, KC * WA], f16, tag="w1")
            nc.sync.dma_start(out=W1_sb[:], in_=t_W1ex[:, :])
            iota_sb = cpool.tile([128, 128], f16, tag="iota")
            nc.sync.dma_start(out=iota_sb[:], in_=t_iota[:, :])
            b1_sb = cpool.tile([128, HC], f32, tag="b1")
            nc.sync.dma_start(out=b1_sb[:], in_=t_b1b[:, :])
            W2_sb = cpool.tile([HC, HC], f16, tag="w2")
            nc.sync.dma_start(out=W2_sb[:], in_=t_W2[:, :])
            vavd_sb = cpool.tile([HC, 2 * H], f16, tag="vavd")
            nc.sync.dma_start(out=vavd_sb[:], in_=t_vavd[:, :])
            b2_sb = cpool.tile([128, C], f32, tag="b2")
            nc.sync.dma_start(out=b2_sb[:], in_=t_b2b[:, :])
            ident = cpool.tile([128, 128], f16, tag="ident")
            make_identity(nc, ident[:])

            # internal DRAM
            h1own = dram.tile([OWN_PAD, W], f16, tag="h1own")
            adst1 = dram.tile([OWN_PAD, H], f16, tag="adst1")
            table1 = dram.tile([NP, W], f16, tag="table1",
                               addr_space="Shared")
            h2own = dram.tile([OWN_PAD, W], f16, tag="h2own")
            adst2 = dram.tile([OWN_PAD, H], f16, tag="adst2")
            table2 = dram.tile([NP, W], f16, tag="table2",
                               addr_space="Shared")

            def all_gather(src_tab, dst_tab, half):
                nc.gpsimd.collective_compute(
                    "AllGather",
                    mybir.AluOpType.bypass,
                    replica_groups=[list(range(NC))],
                    ins=[src_tab[half * ROWS_H:(half + 1) * ROWS_H, :].opt()],
                    outs=[dst_tab[half * NC * ROWS_H:
                                  (half + 1) * NC * ROWS_H, :].opt()],
                )

            # ---------------- phase A: own-shard table1 -------------------
            with (
                tc.tile_pool(name="pa", bufs=4) as pa,
                tc.tile_pool(name="pa_ps", bufs=2, space="PSUM") as pa_ps,
            ):
                for i in range(BLK):
                    xt = pa.tile([128, KC * 128], f16, tag="xt")
                    for q in range(4):   # 4 DMA rings in parallel
                        nc.sync.dma_start(
                            out=xt[q * 32:(q + 1) * 32, :],
                            in_=t_xTc[q * 32:(q + 1) * 32,
                                      i * KC * 128:(i + 1) * KC * 128])
                    ph = pa_ps.tile([128, WA], f32, tag="ph", padded_shape=[128, 512])
                    for k in range(KC):
                        nc.tensor.matmul(
                            out=ph[:],
                            lhsT=xt[:, k * 128:(k + 1) * 128],
                            rhs=W1_sb[:, k * WA:(k + 1) * WA],
                            start=(k == 0), stop=(k == KC - 1))
                    hb = pa.tile([128, WA], f16, tag="hb")
                    nc.scalar.copy(out=hb[:], in_=ph[:])
                    nc.sync.dma_start(
                        out=h1own[i * 128:(i + 1) * 128, :], in_=hb[:, :W])
                    nc.sync.dma_start(
                        out=adst1[i * 128:(i + 1) * 128, :],
                        in_=hb[:, W:WA])
                    if (i + 1) % (BLK // HALVES) == 0:
                        all_gather(h1own, table1,
                                   (i + 1) // (BLK // HALVES) - 1)

            # ---------------- edge phase (shared between layers) ----------
            def edge_phase(tag, tab, adst_tab, finish, mid_hook=None):
                with (
                    tc.tile_pool(name=f"eg{tag}", bufs=2) as eg,
                    tc.tile_pool(name=f"ew{tag}", bufs=2) as ew,
                    tc.tile_pool(name=f"ef{tag}", bufs=2) as ef,
                    tc.tile_pool(name=f"eps{tag}", bufs=2, space="PSUM") as eps,
                    tc.tile_pool(name=f"fps{tag}", bufs=2, space="PSUM") as fps,
                ):
                    for sb in range(BLK // SB):
                        b0 = sb * SB
                        s0 = b0 * NG
                        Tt = sum(Ts[s0:s0 + SB * NG])
                        o = off[s0]
                        idx = eg.tile([128, Tt], i32, tag="idx")
                        nc.sync.dma_start(out=idx[:], in_=t_srcp[:, o:o + Tt])
                        didx = eg.tile([128, Tt], i32, tag="didx")
                        nc.sync.dma_start(out=didx[:],
                                          in_=t_dstl[:, o:o + Tt])
                        slot = eg.tile([128, Tt], f16, tag="slot")
                        nc.sync.dma_start(out=slot[:],
                                          in_=t_slot[:, o:o + Tt])

                        gath = eg.tile([128, Tt * W], f16, tag="gath")
                        nc.gpsimd.indirect_dma_start(
                            out=gath[:], out_offset=None,
                            in_=tab[:, :],
                            in_offset=bass.IndirectOffsetOnAxis(
                                ap=idx[:, :], axis=0))
                        gd = eg.tile([128, Tt * H], f16, tag="gd")
                        nc.gpsimd.indirect_dma_start(
                            out=gd[:], out_offset=None,
                            in_=adst_tab[:, :],
                            in_offset=bass.IndirectOffsetOnAxis(
                                ap=didx[:, :], axis=0))

                        # one-hot Pm[e, (j, d)] = (slot[e, j] == d)
                        Pm = ew.tile([128, Tt * GRP], f16, tag="Pm")
                        nc.vector.tensor_tensor(
                            out=fv(Pm[:], [[GRP, Tt], [1, GRP]]),
                            in0=fv(slot[:], [[1, Tt], [0, GRP]]),
                            in1=fv(iota_sb[:], [[0, Tt], [1, GRP]]),
                            op=mybir.AluOpType.is_equal)

                        # coef = exp(lrelu(a_src[src] + a_dst[dst]))
                        ax = ef.tile([128, Tt * H], f16, tag="ax")
                        nc.vector.tensor_add(
                            out=ax[:],
                            in0=fv(gath[:], [[W, Tt], [1, H]],
                                   extra_offset=HC),
                            in1=gd[:])
                        lr = ef.tile([128, Tt * H], f16, tag="lr")
                        nc.scalar.mul(out=lr[:], in_=ax[:], mul=NEG_SLOPE)
                        nc.vector.tensor_max(out=lr[:], in0=ax[:], in1=lr[:])
                        ex = ef.tile([128, Tt * H], f16, tag="ex")
                        nc.scalar.activation(out=ex[:], in_=lr[:],
                                             func=AF.Exp)

                        # wt = [h * coef (c-major) | coef] per tile: [Tt, 136]
                        wt = ew.tile([128, Tt * W], f16, tag="wt")
                        nc.vector.tensor_copy(
                            out=fv(wt[:], [[W, Tt], [1, H]],
                                   extra_offset=HC),
                            in_=ex[:])
                        exb = ef.tile([128, Tt * HC], f16, tag="exb")
                        nc.vector.tensor_copy(
                            out=fv(exb[:], [[HC, Tt], [H, C], [1, H]]),
                            in_=fv(ex[:], [[H, Tt], [0, C], [1, H]]))
                        nc.vector.tensor_mul(
                            out=fv(wt[:], [[W, Tt], [1, HC]]),
                            in0=fv(gath[:], [[W, Tt], [1, HC]]),
                            in1=exb[:])

                        for bl in range(SB):
                            b = b0 + bl
                            # one PSUM tile per 64-row group: a single
                            # accumulation chain per tensor (two chains on
                            # one tile race in dependency tracking)
                            nd = eps.tile([128, W], f32, tag="nd", padded_shape=[128, 512])
                            nd2 = eps.tile([128, W], f32, tag="nd2", padded_shape=[128, 512])
                            for g in range(NG):
                                s = b * NG + g
                                lb = off[s] - o
                                T = Ts[s]
                                tgt = nd if g == 0 else nd2
                                for j in range(T):
                                    nc.tensor.matmul(
                                        out=tgt[g * GRP:(g + 1) * GRP, :],
                                        lhsT=Pm[:, (lb + j) * GRP:
                                                (lb + j + 1) * GRP],
                                        rhs=wt[:, (lb + j) * W:
                                               (lb + j + 1) * W],
                                        start=(j == 0), stop=(j == T - 1))
                            nc.vector.tensor_copy(
                                out=nd[GRP:128, :], in_=nd2[GRP:128, :])
                            finish(b, nd, ef, fps)
                            if mid_hook is not None:
                                mid_hook(b)

            # ---------------- layer-1 block finisher ----------------------
            def finish1(b, nd, ef, fps):
                dr = ef.tile([128, H], f32, tag="dr")
                nc.vector.tensor_scalar_add(dr[:], nd[:, HC:W], EPS)
                nc.vector.reciprocal(out=dr[:], in_=dr[:])
                g = ef.tile([128, HC], f32, tag="g")
                nc.vector.tensor_tensor(
                    out=fv(g[:], [[H, C], [1, H]]),
                    in0=fv(nd[:], [[H, C], [1, H]]),
                    in1=fv(dr[:], [[0, C], [1, H]]),
                    op=mybir.AluOpType.mult)
                nc.vector.tensor_add(out=g[:], in0=g[:], in1=b1_sb[:])
                # ELU
                tn = ef.tile([128, HC], f32, tag="tn")
                nc.vector.tensor_scalar_min(tn[:], g[:], 0.0)
                te = ef.tile([128, HC], f32, tag="te")
                nc.scalar.activation(out=te[:], in_=tn[:], func=AF.Exp)
                nc.vector.tensor_scalar(
                    out=g[:], in0=g[:], scalar1=0.0, scalar2=-1.0,
                    op0=mybir.AluOpType.max, op1=mybir.AluOpType.add)
                gh = ef.tile([128, HC], f16, tag="gh")
                nc.vector.tensor_add(out=gh[:], in0=g[:], in1=te[:])
                # a_src2 / a_dst2 via g @ (W2 @ att2): needs gT as lhsT
                gtp = fps.tile([128, 128], f16, tag="gtp", padded_shape=[128, 1024])
                nc.tensor.transpose(out=gtp[:], in_=gh[:], identity=ident[:])
                gts = ef.tile([128, 128], f16, tag="gts")
                nc.scalar.copy(out=gts[:], in_=gtp[:])
                a2p = fps.tile([128, 2 * H], f32, tag="a2p", padded_shape=[128, 512])
                nc.tensor.matmul(out=a2p[:], lhsT=gts[:], rhs=vavd_sb[:],
                                 start=True, stop=True)
                a2s = ef.tile([128, 2 * H], f16, tag="a2s")
                nc.scalar.copy(out=a2s[:], in_=a2p[:])
                nc.sync.dma_start(
                    out=h2own[b * 128:(b + 1) * 128, :HC], in_=gh[:])
                nc.sync.dma_start(
                    out=h2own[b * 128:(b + 1) * 128, HC:W], in_=a2s[:, :H])
                nc.sync.dma_start(
                    out=adst2[b * 128:(b + 1) * 128, :], in_=a2s[:, H:])

            # ---------------- layer-2 block finisher ----------------------
            def finish2(b, nd, ef, fps):
                dr = ef.tile([128, H], f32, tag="dr")
                nc.vector.tensor_scalar_add(dr[:], nd[:, HC:W], EPS)
                nc.vector.reciprocal(out=dr[:], in_=dr[:])
                g = ef.tile([128, HC], f16, tag="g")
                nc.vector.tensor_tensor(
                    out=fv(g[:], [[H, C], [1, H]]),
                    in0=fv(nd[:], [[H, C], [1, H]]),
                    in1=fv(dr[:], [[0, C], [1, H]]),
                    op=mybir.AluOpType.mult)
                atp = fps.tile([128, 128], f16, tag="gtp", padded_shape=[128, 1024])
                nc.tensor.transpose(out=atp[:], in_=g[:], identity=ident[:])
                ats = ef.tile([128, 128], f16, tag="gts")
                nc.scalar.copy(out=ats[:], in_=atp[:])
                o2 = fps.tile([128, HC], f32, tag="o2", padded_shape=[128, 512])
                nc.tensor.matmul(out=o2[:], lhsT=ats[:], rhs=W2_sb[:],
                                 start=True, stop=True)
                red = ef.tile([128, C], f32, tag="red")
                nc.vector.reduce_sum(
                    out=red[:], in_=fv(o2[:], [[1, C], [C, H]]), axis=X)
                nc.vector.tensor_scalar_mul(red[:], red[:], 1.0 / H)
                nc.vector.tensor_add(out=red[:], in0=red[:], in1=b2_sb[:])
                # log_softmax over the C classes
                mx = ef.tile([128, 1], f32, tag="mx")
                nc.vector.reduce_max(out=mx[:], in_=red[:], axis=X)
                nc.vector.tensor_sub(
                    out=red[:], in0=red[:], in1=mx[:].to_broadcast([128, C]))
                es = ef.tile([128, C], f32, tag="es")
                nc.scalar.activation(out=es[:], in_=red[:], func=AF.Exp)
                sm = ef.tile([128, 1], f32, tag="sm")
                nc.vector.reduce_sum(out=sm[:], in_=es[:], axis=X)
                ls = ef.tile([128, 1], f32, tag="ls")
                nc.scalar.activation(out=ls[:], in_=sm[:], func=AF.Ln)
                nc.vector.tensor_sub(
                    out=red[:], in0=red[:], in1=ls[:].to_broadcast([128, C]))
                nc.sync.dma_start(
                    out=t_out[b * 128:(b + 1) * 128, :], in_=red[:])

            # ---------------- run both layers ------------------------------
            def hook1(b):
                if (b + 1) % (BLK // HALVES) == 0:
                    all_gather(h2own, table2, (b + 1) // (BLK // HALVES) - 1)

            edge_phase("1", table1, adst1, finish1, mid_hook=hook1)
            edge_phase("2", table2, adst2, finish2)

    nc.compile()
    return nc


# ---------------------------------------------------------------------------
# entry point
# ---------------------------------------------------------------------------

def _run(cfg, inputs, trace=False):
    from concourse.bass_utils import run_bass_kernel_spmd

    cfg = _derive(cfg)
    in_maps, Ts, off, totT = _host_prep(cfg, **inputs)
    nc = build_program(cfg, Ts, off, totT)
    res = run_bass_kernel_spmd(
        nc, in_maps, core_ids=list(range(cfg["NC"])), trace=trace)
    outs = []
    for m in range(cfg["NC"]):
        outs.append(res.results[m]["out"][:cfg["OWN"]])
    full = np.concatenate(outs, axis=0)
    return full, res


def kernel(x, edge_index, W1, att_src1, att_dst1, bias1, W2, att_src2,
           att_dst2, bias2):
    inputs = dict(x=np.asarray(x, np.float32),
                  edge_index=np.asarray(edge_index),
                  W1=W1, att_src1=att_src1, att_dst1=att_dst1, bias1=bias1,
                  W2=W2, att_src2=att_src2, att_dst2=att_dst2, bias2=bias2)
    out, _ = _run(FULL_CFG, inputs, trace=False)
    return out
